# revision 5
# baseline (speedup 1.0000x reference)
"""KV-cache scatter kernel for Trainium2 (8 NeuronCores, batch-sharded).

Problem: k_out = k_cache.at[b, :, input_pos[b, t], :].set(k[b, :, t, :])
         (same for v). Shapes: k/v (B,H,T,D)=(8,16,16,128),
         caches (B,H,S,D)=(8,16,4096,128), input_pos (B,T).

Strategy: shard the batch dim across the 8 cores (one batch row each).
Per core the kernel is memory-bound on the out-of-place cache copy
(2 x 32 MiB DRAM->DRAM), followed by an indirect-DMA scatter of the
256 update rows (H*T) using host-precomputed flat offsets h*S + pos.
"""

import numpy as np

B, H, T, D = 8, 16, 16, 128
S = 4096
HS = H * S  # 65536 rows in the flattened (H*S, D) cache view
NROW = H * T  # 256 update rows per batch element
P = 128  # SBUF partitions

_PROGRAM = None


def _build_program(n_iters=1, n_chunks=4):
    """Build the per-core Bass program.

    n_iters > 1 repeats the whole copy+scatter body serially N times -- used
    only by the timing harness (one bass_exec per XLA module is allowed, so
    repetition has to live inside the program).
    """
    import concourse.bass as bass
    import concourse.mybir as mybir

    dt = mybir.dt
    nc = bass.Bass()

    k_cache_in = nc.declare_dram_parameter("k_cache_in", [HS, D], dt.float32, isOutput=False)
    v_cache_in = nc.declare_dram_parameter("v_cache_in", [HS, D], dt.float32, isOutput=False)
    k_upd = nc.declare_dram_parameter("k_upd", [NROW, D], dt.float32, isOutput=False)
    v_upd = nc.declare_dram_parameter("v_upd", [NROW, D], dt.float32, isOutput=False)
    offsets = nc.declare_dram_parameter("offsets", [NROW, 1], dt.int32, isOutput=False)
    k_out = nc.declare_dram_parameter("k_out", [HS, D], dt.float32, isOutput=True)
    v_out = nc.declare_dram_parameter("v_out", [HS, D], dt.float32, isOutput=True)

    with (
        nc.sbuf_tensor("ku0", [P, D], dt.float32) as ku0,
        nc.sbuf_tensor("ku1", [P, D], dt.float32) as ku1,
        nc.sbuf_tensor("vu0", [P, D], dt.float32) as vu0,
        nc.sbuf_tensor("vu1", [P, D], dt.float32) as vu1,
        nc.sbuf_tensor("off0", [P, 1], dt.int32) as off0,
        nc.sbuf_tensor("off1", [P, 1], dt.int32) as off1,
        nc.semaphore("bulk_sem") as bulk_sem,
        nc.semaphore("ld_sem") as ld_sem,
        nc.semaphore("sc_sem") as sc_sem,
        nc.Block() as block,
    ):
        # Bulk cache copy: DRAM->DRAM, one big transfer per cache, on the
        # two HWDGE rings so descriptor generation runs in parallel.
        # Iteration i's copies wait for iteration i-1's scatters (WAW on the
        # same output rows); within an iteration k and v run concurrently.
        rows = HS // n_chunks

        @block.sync
        def _(sync):
            for i in range(n_iters):
                sync.wait_ge(sc_sem, 64 * i)
                for c in range(n_chunks):
                    sync.dma_start(
                        out=k_out[c * rows : (c + 1) * rows, :],
                        in_=k_cache_in[c * rows : (c + 1) * rows, :],
                    ).then_inc(bulk_sem, 16)

        @block.scalar
        def _(scalar):
            for i in range(n_iters):
                scalar.wait_ge(sc_sem, 64 * i)
                for c in range(n_chunks):
                    scalar.dma_start(
                        out=v_out[c * rows : (c + 1) * rows, :],
                        in_=v_cache_in[c * rows : (c + 1) * rows, :],
                    ).then_inc(bulk_sem, 16)

        @block.gpsimd
        def _(g):
            loads = [
                (off0[:, :], offsets[0:P, :]),
                (off1[:, :], offsets[P:NROW, :]),
                (ku0[:, :], k_upd[0:P, :]),
                (ku1[:, :], k_upd[P:NROW, :]),
                (vu0[:, :], v_upd[0:P, :]),
                (vu1[:, :], v_upd[P:NROW, :]),
            ]
            for dst, src in loads:
                g.dma_start(out=dst, in_=src).then_inc(ld_sem, 16)
            g.wait_ge(ld_sem, 16 * len(loads))
            scatters = [
                (k_out, off0, ku0),
                (k_out, off1, ku1),
                (v_out, off0, vu0),
                (v_out, off1, vu1),
            ]
            for i in range(n_iters):
                # Scatter rows must land after this iteration's bulk copy.
                g.wait_ge(bulk_sem, 32 * n_chunks * (i + 1))
                for out_t, off_t, src_t in scatters:
                    g.indirect_dma_start(
                        out=out_t[:, :],
                        out_offset=bass.IndirectOffsetOnAxis(ap=off_t[:, :1], axis=0),
                        in_=src_t[:, :],
                        in_offset=None,
                    ).then_inc(sc_sem, 16)
            g.wait_ge(sc_sem, 64 * n_iters)

    return nc


def _make_in_maps(input_pos, k, v, k_cache, v_cache):
    input_pos = np.asarray(input_pos)
    k = np.ascontiguousarray(np.asarray(k, dtype=np.float32))
    v = np.ascontiguousarray(np.asarray(v, dtype=np.float32))
    k_cache = np.ascontiguousarray(np.asarray(k_cache, dtype=np.float32))
    v_cache = np.ascontiguousarray(np.asarray(v_cache, dtype=np.float32))

    h_off = np.arange(H, dtype=np.int64)[:, None] * S  # (H, 1)
    in_maps = []
    for b in range(B):
        pos_b = input_pos[b].astype(np.int64)  # (T,)
        offs = (h_off + pos_b[None, :]).reshape(NROW, 1).astype(np.int32)
        in_maps.append(
            {
                "k_cache_in": k_cache[b].reshape(HS, D),
                "v_cache_in": v_cache[b].reshape(HS, D),
                "k_upd": k[b].reshape(NROW, D),
                "v_upd": v[b].reshape(NROW, D),
                "offsets": offs,
            }
        )
    return in_maps


def run_with_results(input_pos, k, v, k_cache, v_cache, trace=False):
    """Run on the 8 cores; returns ((k_out, v_out), BassKernelResults)."""
    global _PROGRAM
    import os

    # NTFF tracing is unavailable under axon in this container (no
    # antenv.axon_hooks); make sure an inherited BASS_TRACE can't divert
    # run_bass_kernel_spmd into the crashing trace path.
    if not trace:
        os.environ["BASS_NEVER_TRACE"] = "1"
    from concourse.bass_utils import run_bass_kernel_spmd

    if _PROGRAM is None:
        _PROGRAM = _build_program()

    in_maps = _make_in_maps(input_pos, k, v, k_cache, v_cache)
    res = run_bass_kernel_spmd(_PROGRAM, in_maps, core_ids=list(range(B)), trace=trace)
    k_out = np.stack([np.asarray(res.results[b]["k_out"]).reshape(H, S, D) for b in range(B)])
    v_out = np.stack([np.asarray(res.results[b]["v_out"]).reshape(H, S, D) for b in range(B)])
    return (k_out, v_out), res


def kernel(input_pos, k, v, k_cache, v_cache):
    (k_out, v_out), _ = run_with_results(input_pos, k, v, k_cache, v_cache)
    return k_out, v_out
